# revision 5
# baseline (speedup 1.0000x reference)
"""SimCLR-style contrastive loss (nn_Contrast) on 8 Trainium2 NeuronCores.

Symmetry-exploiting data-parallel scheme:
  z = concat(normalize(x_i), normalize(x_j)) has a SYMMETRIC sim matrix
  sim = (z @ z.T)/TEMP, so each (i,j) block only needs to be computed once
  globally.  Rows are split into 16 strips of 512; the core owning strips
  (2c, 2c+1) (host pre-rotates z by -c*1024 so every core sees its strips
  as local strips 0,1 -- one SPMD program) computes, for each of its
  strips s, the blocks (s, s+k) for k=0..8, with the k=8 block
  half-weighted (exp bias = -ln2; the distance-8 pair is computed by both
  covering cores).  Per core that is 2*8.5 = 17 block-equivalents of
  512x512 instead of the 64 a full slab costs -- 47% less exp/matmul work.

  Row sums of exp come free via the activation's accum_out.  The missing
  contributions (each block also serves the rows of its COLUMN strip) are
  computed as column sums on the PE: the exp'd block is written to SBUF
  (fp8) and contracted against one-hot ones-vectors with a DoubleRow fp8
  matmul into a [8, 512] PSUM tile (partition = k), DMA'd out, and the
  host adds them into the right global strips.

  Device outputs per core: s_out [128,8] row-sum partials, scol [16,512]
  column-sum vectors (2 strips x k=1..8), p2_out [128,8] positive-pair
  logits.  Host: S = scatter(srow) + scatter(scol); loss =
  mean(-p2 + log(S - e^2 + exp(p2))).
"""

import numpy as np

B = 4096
D = 256
NB = 2 * B              # 8192 rows of z
NCORES = 8
SLAB = NB // NCORES     # 1024 rows per core
STRIP = 512
NSTRIP = NB // STRIP    # 16 strips globally
KDIST = 8               # max block distance (k=8 half-weighted)
NLOAD_S = 2 + KDIST     # strips loaded per core (own 2 + cols up to k=8 from strip 1)
NLOAD_T = NLOAD_S * 4   # 40 row tiles of 128
JSPAN = (KDIST + 1) * STRIP  # 4608 j-columns per strip
NCHUNK = 5              # j-chunks per strip: 4x1024 + 1x512
TEMP = 0.5
INV_TEMP = 1.0 / TEMP
E2 = float(np.exp(INV_TEMP))
LN_HALF = float(np.log(0.5))

_nc_cache = None


def _patch_tile_drain():
    """This container's walrus accepts at most ONE sem-wait per instruction,
    but Tile's wait assignment can attach several (and the tail drain gets
    one per busy proc).  Legalize by hoisting extra waits onto preceding
    same-engine NoOps (same semantics: an engine executes its stream in
    order, and multi-waits are AND conditions)."""
    import concourse.tile as tile
    from concourse import mybir
    from concourse.vector_clock import ScopedClock

    if getattr(tile.TileContext, "_drain_patch_applied", False):
        return

    _ctr = [0]

    def _legalize_waits(nc):
        for f in nc.m.functions:
            for bb in f.blocks:
                insts = bb.instructions
                new = []
                changed = False
                for inst in insts:
                    si = inst.sync_info
                    waits = list(si.on_wait) if (si and si.on_wait) else []
                    if len(waits) > 1:
                        for w in waits[:-1]:
                            _ctr[0] += 1
                            nop = mybir.InstNoOp(
                                name=f"legalize-wait-{_ctr[0]}", ins=[], outs=[]
                            )
                            nop.engine = inst.engine
                            nop.sync_info = mybir.SyncInfo(
                                on_wait=[w], on_update=[]
                            )
                            new.append(nop)
                        si.on_wait = [waits[-1]]
                        changed = True
                    new.append(inst)
                if changed:
                    bb.instructions = new

    def _drain_and_barrier(self, tick_clock, wait_clock):
        nc = self.nc
        nop0 = nc.sync.nop()
        wait_clock.add_sem_waits(
            nop0.ins, ScopedClock({None: tick_clock.global_clock})
        )
        nc.sync.drain()
        nc.all_engine_barrier()
        assert self.sems is not None
        popped = nc._tile_sem_poison_stack.pop()
        assert popped is self._sem_poison
        nc.clear_and_free_semaphores(list(self.sems.allocated().values()))
        nc.all_engine_barrier()
        _legalize_waits(nc)

    tile.TileContext._drain_and_barrier = _drain_and_barrier
    tile.TileContext._drain_patch_applied = True


def _build_nc(repeat=1, exp_fp8=True):
    from concourse import mybir, masks
    import concourse.bass as bass
    import concourse.tile as tile
    import contextlib

    _patch_tile_drain()

    f32 = mybir.dt.float32
    bf16 = mybir.dt.bfloat16
    fp8 = mybir.dt.float8e4
    expdt = fp8 if exp_fp8 else bf16
    Act = mybir.ActivationFunctionType
    Alu = mybir.AluOpType
    DR = mybir.MatmulPerfMode.DoubleRow

    nc = bass.Bass()
    z_dram = nc.dram_tensor("z", [NLOAD_S * STRIP, D], f32, kind="ExternalInput")
    s_dram = nc.dram_tensor("s_out", [128, 8], f32, kind="ExternalOutput")
    p2_dram = nc.dram_tensor("p2_out", [128, 8], f32, kind="ExternalOutput")
    scol_dram = nc.dram_tensor("scol_out", [8, 2, 512], f32, kind="ExternalOutput")

    with tile.TileContext(nc) as tc:
        rep_ctx = tc.For_i(0, repeat) if repeat > 1 else contextlib.nullcontext()
        with (
            rep_ctx,
            tc.tile_pool(name="persist", bufs=1) as persist,
            tc.tile_pool(name="scratch", bufs=4) as scratch,
            tc.tile_pool(name="zbfp", bufs=2) as zbfp,
            tc.tile_pool(name="exppool", bufs=2) as exppool,
            tc.tile_pool(name="psum", bufs=2, space="PSUM") as psum,
            tc.tile_pool(name="psum_tp", bufs=2, space="PSUM") as psum_tp,
            tc.tile_pool(name="psum_col", bufs=2, space="PSUM") as psum_col,
        ):
            zraw = persist.tile([128, NLOAD_T, D], f32, tag="zraw")
            zT = [
                persist.tile([128, NLOAD_S * STRIP], bf16, tag="zT0", name="zT0"),
                persist.tile([128, NLOAD_S * STRIP], bf16, tag="zT1", name="zT1"),
            ]
            norms2 = persist.tile([128, NLOAD_T], f32, tag="norms2")
            lnb = persist.tile([128, NLOAD_T], f32, tag="lnb")
            rinorm = persist.tile([128, NLOAD_T], f32, tag="rinorm")
            accum = persist.tile([128, 8 * NCHUNK], f32, tag="accum")
            s_tile = persist.tile([128, 8], f32, tag="s_tile")
            dotraw = persist.tile([128, 8], f32, tag="dotraw")
            tmp8 = persist.tile([128, 8], f32, tag="tmp8")
            pos2 = persist.tile([128, 8], f32, tag="pos2")
            ident = persist.tile([128, 128], bf16, tag="ident")
            colsb = persist.tile([8, 2, 512], f32, tag="colsb")
            biasln = persist.tile([128, 1], f32, tag="biasln")
            # one-hot ones weights for column-sum matmuls: [kt, idx, m]
            oh = persist.tile([128, 2, KDIST, KDIST], expdt, tag="oh")

            masks.make_identity(nc, ident[:])
            nc.vector.memset(biasln, LN_HALF)
            nc.vector.memset(oh, 0.0)
            for kt in range(2):
                for idx in range(KDIST):
                    nc.vector.memset(oh[:, kt, idx, idx : idx + 1], 1.0)

            # ---------------- load-phase helpers ----------------
            def load_strip(s):
                for k in range(4):
                    t = s * 4 + k
                    nc.sync.dma_start(
                        out=zraw[:, t, :], in_=z_dram[t * 128 : (t + 1) * 128, :]
                    )
                    sq = scratch.tile([128, D], f32, tag="sq_scratch")
                    nc.vector.scalar_tensor_tensor(
                        out=sq,
                        in0=zraw[:, t, :],
                        scalar=1.0,
                        in1=zraw[:, t, :],
                        op0=Alu.mult,
                        op1=Alu.mult,
                        accum_out=norms2[:, t : t + 1],
                    )

            def norm_pair(g):
                # rinorm = exp(-0.5 * ln(sumsq)) for 8 tiles (strips 2g, 2g+1)
                gs = slice(g * 8, g * 8 + 8)
                nc.scalar.activation(out=lnb[:, gs], in_=norms2[:, gs], func=Act.Ln)
                nc.scalar.activation(
                    out=rinorm[:, gs], in_=lnb[:, gs], func=Act.Exp, scale=-0.5
                )

            def fin_strip(s):
                zbf = zbfp.tile([128, 4, D], bf16, tag="zbf")
                for k in range(4):
                    t = s * 4 + k
                    nc.vector.tensor_scalar_mul(
                        zbf[:, k, :], zraw[:, t, :], rinorm[:, t : t + 1]
                    )
                tp = psum_tp.tile([128, 1024], bf16, tag="tp")
                for d in range(2):
                    for k in range(4):
                        nc.tensor.transpose(
                            tp[:, (d * 4 + k) * 128 : (d * 4 + k + 1) * 128],
                            zbf[:, k, d * 128 : (d + 1) * 128],
                            ident,
                        )
                for d in range(2):
                    nc.vector.tensor_copy(
                        zT[d][:, s * 512 : s * 512 + 512],
                        tp[:, d * 512 : (d + 1) * 512],
                    )

            # ---------------- main-phase helpers ----------------
            strip_state = {}

            def main_batch(sl, c):
                W = 1024 if c < 4 else 512
                jb = sl * 512 + c * 1024
                if c == 0:
                    et = exppool.tile([128, 4, JSPAN], expdt, tag="exp8")
                    cp = psum_col.tile([8, 512], f32, tag="colps")
                    strip_state[sl] = (et, cp)
                et, cp = strip_state[sl]

                def colsums(pair):
                    ks = [2 * c, 2 * c + 1] if c < 4 else [KDIST]
                    for k in ks:
                        if k == 0:
                            continue
                        first = (c == 0 and k == 1 and pair == 0)
                        last = (c == 4 and k == KDIST and pair == 1)
                        if exp_fp8:
                            nc.tensor.matmul(
                                cp[:, :],
                                lhsT=oh[:, :, k - 1, :],
                                rhs=et[:, 2 * pair : 2 * pair + 2,
                                       k * 512 : (k + 1) * 512],
                                start=first,
                                stop=last,
                                perf_mode=DR,
                                skip_group_check=True,
                            )
                        else:
                            for sub in range(2):
                                it2 = 2 * pair + sub
                                nc.tensor.matmul(
                                    cp[:, :],
                                    lhsT=oh[:, 0, k - 1, :],
                                    rhs=et[:, it2, k * 512 : (k + 1) * 512],
                                    start=first and sub == 0,
                                    stop=last and sub == 1,
                                    skip_group_check=True,
                                )

                for it in range(4):
                    ig = sl * 4 + it
                    pt = psum.tile([128, 1024], f32, tag="pt")
                    for d in range(2):
                        for jc in range(W // 512):
                            j0 = jb + jc * 512
                            nc.tensor.matmul(
                                pt[:, jc * 512 : (jc + 1) * 512],
                                lhsT=zT[d][:, ig * 128 : (ig + 1) * 128],
                                rhs=zT[d][:, j0 : j0 + 512],
                                start=(d == 0),
                                stop=(d == 1),
                            )
                    nc.scalar.activation(
                        out=et[:, it, c * 1024 : c * 1024 + W],
                        in_=pt[:, 0:W],
                        func=Act.Exp,
                        scale=float(INV_TEMP),
                        bias=(biasln[:, :] if c == 4 else 0.0),
                        accum_out=accum[:, ig * NCHUNK + c : ig * NCHUNK + c + 1],
                    )
                    if it == 1:
                        colsums(0)
                    elif it == 3:
                        colsums(1)
                if c == 4:
                    # strip finished: stage colsums to SBUF (DMA can't read PSUM)
                    nc.vector.tensor_copy(colsb[:, sl, :], cp[:, :])

            def pos_pairs():
                # positive pairs: raw dot of slab rows (tiles 0..7) with their
                # partner rows at +B (tiles 32..39)
                for t in range(8):
                    pscr = scratch.tile([128, D], f32, tag="sq_scratch")
                    nc.vector.scalar_tensor_tensor(
                        out=pscr,
                        in0=zraw[:, t, :],
                        scalar=1.0,
                        in1=zraw[:, t + 32, :],
                        op0=Alu.mult,
                        op1=Alu.mult,
                        accum_out=dotraw[:, t : t + 1],
                    )
                nc.vector.tensor_mul(tmp8, rinorm[:, 0:8], rinorm[:, 32:40])
                nc.vector.scalar_tensor_tensor(
                    out=pos2,
                    in0=dotraw,
                    scalar=float(INV_TEMP),
                    in1=tmp8,
                    op0=Alu.mult,
                    op1=Alu.mult,
                )
                nc.sync.dma_start(out=p2_dram[:, :], in_=pos2)

            # ---------------- emission schedule ----------------
            load_strip(0)
            load_strip(1)
            norm_pair(0)
            fin_strip(0)
            fin_strip(1)

            sched = [
                ("L", 2), ("L", 3), ("N", 1), ("F", 2), ("F", 3),
                ("B", 0, 0), ("B", 1, 0),
                ("L", 4), ("L", 5), ("N", 2), ("F", 4), ("F", 5),
                ("B", 0, 1), ("B", 1, 1),
                ("L", 6), ("L", 7), ("N", 3), ("F", 6), ("F", 7),
                ("B", 0, 2), ("B", 1, 2),
                ("L", 8), ("L", 9), ("N", 4), ("F", 8), ("F", 9),
                ("B", 0, 3), ("B", 1, 3),
                ("P",),
                ("B", 0, 4), ("B", 1, 4),
            ]
            for item in sched:
                if item[0] == "L":
                    load_strip(item[1])
                elif item[0] == "N":
                    norm_pair(item[1])
                elif item[0] == "F":
                    fin_strip(item[1])
                elif item[0] == "B":
                    main_batch(item[1], item[2])
                elif item[0] == "P":
                    pos_pairs()

            # S[:, ig] = sum of the NCHUNK partial row-sums
            nc.vector.tensor_reduce(
                out=s_tile,
                in_=accum.rearrange("p (a b) -> p a b", b=NCHUNK),
                axis=mybir.AxisListType.X,
                op=Alu.add,
            )
            nc.sync.dma_start(out=s_dram[:, :], in_=s_tile)
            nc.sync.dma_start(out=scol_dram[:, :, :], in_=colsb)

    return nc


def _get_nc():
    global _nc_cache
    if _nc_cache is None:
        _nc_cache = _build_nc()
    return _nc_cache


def kernel(x_i, x_j):
    from concourse import bass_utils

    z = np.concatenate(
        [np.asarray(x_i, dtype=np.float32), np.asarray(x_j, dtype=np.float32)], axis=0
    )
    in_maps = [
        {"z": np.ascontiguousarray(
            np.roll(z, -c * SLAB, axis=0)[: NLOAD_S * STRIP])}
        for c in range(NCORES)
    ]
    nc = _get_nc()
    res = bass_utils.run_bass_kernel_spmd(nc, in_maps, core_ids=list(range(NCORES)))

    S_glob = np.zeros(NB, dtype=np.float64)
    P2_glob = np.zeros(NB, dtype=np.float64)
    for c in range(NCORES):
        srow = np.asarray(res.results[c]["s_out"], dtype=np.float64)   # [128, 8]
        p2 = np.asarray(res.results[c]["p2_out"], dtype=np.float64)    # [128, 8]
        scol = np.asarray(res.results[c]["scol_out"], dtype=np.float64)  # [8, 2, 512]
        base = c * SLAB
        S_glob[base : base + SLAB] += srow.T.reshape(SLAB)
        P2_glob[base : base + SLAB] = p2.T.reshape(SLAB)
        for sl in range(2):
            for k in range(1, KDIST + 1):
                t = (2 * c + sl + k) % NSTRIP
                S_glob[t * STRIP : (t + 1) * STRIP] += scol[k - 1, sl]

    loss = -P2_glob + np.log(S_glob - E2 + np.exp(P2_glob))
    return np.array(loss.mean(), dtype=np.float32)


# revision 18
# speedup vs baseline: 1.2582x; 1.2582x over previous
"""SimCLR-style contrastive loss (nn_Contrast) on 8 Trainium2 NeuronCores.

Symmetry-exploiting data-parallel scheme:
  z = concat(normalize(x_i), normalize(x_j)) has a SYMMETRIC sim matrix
  sim = (z @ z.T)/TEMP, so each (i,j) block only needs to be computed once
  globally.  Rows are split into 16 strips of 512; the core owning strips
  (2c, 2c+1) (host pre-rotates z by -c*1024 so every core sees its strips
  as local strips 0,1 -- one SPMD program) computes, for each of its
  strips s, the blocks (s, s+k) for k=0..8, with the k=8 block
  half-weighted (exp bias = -ln2; the distance-8 pair is computed by both
  covering cores).  Per core that is 2*8.5 = 17 block-equivalents of
  512x512 instead of the 64 a full slab costs -- 47% less exp/matmul work.

  Row sums of exp come free via the activation's accum_out.  The missing
  contributions (each block also serves the rows of its COLUMN strip) are
  computed as column sums on the PE: the exp'd block is written to SBUF
  (fp8) and contracted against one-hot ones-vectors with a DoubleRow fp8
  matmul into a [8, 512] PSUM tile (partition = k), DMA'd out, and the
  host adds them into the right global strips.

  Device outputs per core: s_out [128,8] row-sum partials, scol [16,512]
  column-sum vectors (2 strips x k=1..8), p2_out [128,8] positive-pair
  logits.  Host: S = scatter(srow) + scatter(scol); loss =
  mean(-p2 + log(S - e^2 + exp(p2))).
"""

import numpy as np
import ml_dtypes

B = 4096
D = 256
NB = 2 * B              # 8192 rows of z
NCORES = 8
SLAB = NB // NCORES     # 1024 rows per core
STRIP = 512
NSTRIP = NB // STRIP    # 16 strips globally
KDIST = 8               # max block distance (k=8 half-weighted)
NLOAD_S = 2 + KDIST     # strips loaded per core (own 2 + cols up to k=8 from strip 1)
NLOAD_T = NLOAD_S * 4   # 40 row tiles of 128
JSPAN = (KDIST + 1) * STRIP  # 4608 j-columns per strip
NCHUNK = 3              # j-chunks per strip: 3 x 1536
CHUNKS = [(0, 1536), (1536, 1536), (3072, 1536)]
CHUNK_KS = [(1, 2), (3, 4, 5), (6, 7)]  # k-strips needing colsums per chunk
# (k=8 blocks are computed redundantly by both covering cores at full weight;
#  each strip's k=8 contribution comes from its own rowsums, so no k=8
#  colsums and no half-weighting are needed.)
ZSCALE = 16.0           # fp8 pre-scale of normalized z rows
LN_ZSCALE = float(np.log(ZSCALE))
TEMP = 0.5
INV_TEMP = 1.0 / TEMP
E2 = float(np.exp(INV_TEMP))

_nc_cache = None


def _patch_tile_drain():
    """This container's walrus accepts at most ONE sem-wait per instruction,
    but Tile's wait assignment can attach several (and the tail drain gets
    one per busy proc).  Legalize by hoisting extra waits onto preceding
    same-engine NoOps (same semantics: an engine executes its stream in
    order, and multi-waits are AND conditions)."""
    import concourse.tile as tile
    from concourse import mybir
    from concourse.vector_clock import ScopedClock

    if getattr(tile.TileContext, "_drain_patch_applied", False):
        return

    _ctr = [0]

    def _legalize_waits(nc):
        for f in nc.m.functions:
            for bb in f.blocks:
                insts = bb.instructions
                new = []
                changed = False
                for inst in insts:
                    si = inst.sync_info
                    waits = list(si.on_wait) if (si and si.on_wait) else []
                    if len(waits) > 1:
                        for w in waits[:-1]:
                            _ctr[0] += 1
                            nop = mybir.InstNoOp(
                                name=f"legalize-wait-{_ctr[0]}", ins=[], outs=[]
                            )
                            nop.engine = inst.engine
                            nop.sync_info = mybir.SyncInfo(
                                on_wait=[w], on_update=[]
                            )
                            new.append(nop)
                        si.on_wait = [waits[-1]]
                        changed = True
                    new.append(inst)
                if changed:
                    bb.instructions = new

    def _drain_and_barrier(self, tick_clock, wait_clock):
        nc = self.nc
        nop0 = nc.sync.nop()
        wait_clock.add_sem_waits(
            nop0.ins, ScopedClock({None: tick_clock.global_clock})
        )
        nc.sync.drain()
        nc.all_engine_barrier()
        assert self.sems is not None
        popped = nc._tile_sem_poison_stack.pop()
        assert popped is self._sem_poison
        nc.clear_and_free_semaphores(list(self.sems.allocated().values()))
        nc.all_engine_barrier()
        _legalize_waits(nc)

    tile.TileContext._drain_and_barrier = _drain_and_barrier
    tile.TileContext._drain_patch_applied = True


def _build_nc(repeat=1, exp_fp8=True):
    from concourse import mybir, masks
    import concourse.bass as bass
    import concourse.tile as tile
    import contextlib

    _patch_tile_drain()

    f32 = mybir.dt.float32
    bf16 = mybir.dt.bfloat16
    fp8 = mybir.dt.float8e4
    expdt = fp8 if exp_fp8 else bf16
    Act = mybir.ActivationFunctionType
    Alu = mybir.AluOpType
    DR = mybir.MatmulPerfMode.DoubleRow

    nc = bass.Bass()
    z_dram = nc.dram_tensor("z", [NLOAD_S * STRIP, D], bf16, kind="ExternalInput")
    s_dram = nc.dram_tensor("s_out", [128, 8 * NCHUNK], f32, kind="ExternalOutput")
    p2_dram = nc.dram_tensor("p2_out", [128, 8], f32, kind="ExternalOutput")
    scol_dram = nc.dram_tensor("scol_out", [8, 2, 512], f32, kind="ExternalOutput")

    with tile.TileContext(nc) as tc:
        rep_ctx = tc.For_i(0, repeat) if repeat > 1 else contextlib.nullcontext()
        with (
            rep_ctx,
            tc.tile_pool(name="persist", bufs=1) as persist,
            tc.tile_pool(name="scratch", bufs=4) as scratch,
            tc.tile_pool(name="zbfp", bufs=2) as zbfp,
            tc.tile_pool(name="exppool", bufs=2) as exppool,
            tc.tile_pool(name="psum", bufs=2, space="PSUM") as psum,
            tc.tile_pool(name="psum_tp", bufs=2, space="PSUM") as psum_tp,
        ):
            zraw = persist.tile([128, NLOAD_T, D], bf16, tag="zraw")
            zT8 = persist.tile([128, 2, NLOAD_S * STRIP], fp8, tag="zT8")
            norms2 = persist.tile([128, NLOAD_T], f32, tag="norms2")
            lnb = persist.tile([128, NLOAD_T], f32, tag="lnb")
            rinorm = persist.tile([128, NLOAD_T], f32, tag="rinorm")
            accum = persist.tile([128, 8 * NCHUNK], f32, tag="accum")
            dotraw = persist.tile([128, 8], f32, tag="dotraw")
            tmp8 = persist.tile([128, 8], f32, tag="tmp8")
            pos2 = persist.tile([128, 8], f32, tag="pos2")
            ident = persist.tile([128, 128], bf16, tag="ident")
            colsb = persist.tile([8, 2, 512], f32, tag="colsb")
            bln16 = persist.tile([128, 1], f32, tag="bln16")
            # one-hot ones weights for column-sum matmuls: [kt, idx, m]
            oh = persist.tile([128, 2, KDIST, KDIST], expdt, tag="oh")

            masks.make_identity(nc, ident[:])
            nc.vector.memset(bln16, LN_ZSCALE)
            nc.vector.memset(oh, 0.0)
            for kt in range(2):
                for idx in range(KDIST):
                    nc.vector.memset(oh[:, kt, idx, idx : idx + 1], 1.0)

            # ---------------- load-phase helpers ----------------
            def load_dma(s):
                nc.sync.dma_start(
                    out=zraw[:, s * 4 : (s + 1) * 4, :],
                    in_=z_dram[s * 512 : (s + 1) * 512, :].rearrange(
                        "(t p) d -> p t d", p=128
                    ),
                )

            def strip_sq(s, eng):
                for k in range(4):
                    t = s * 4 + k
                    sq = scratch.tile([128, D], bf16, tag="sq_scratch")
                    eng.scalar_tensor_tensor(
                        out=sq,
                        in0=zraw[:, t, :],
                        scalar=1.0,
                        in1=zraw[:, t, :],
                        op0=Alu.mult,
                        op1=Alu.mult,
                        accum_out=norms2[:, t : t + 1],
                    )

            def norm_strips(t0, t1):
                # rinorm = ZSCALE * exp(-0.5 * ln(sumsq)) for tiles [t0, t1)
                gs = slice(t0, t1)
                nc.scalar.activation(out=lnb[:, gs], in_=norms2[:, gs], func=Act.Ln)
                nc.scalar.activation(
                    out=rinorm[:, gs], in_=lnb[:, gs], func=Act.Exp, scale=-0.5,
                    bias=bln16[:, :],
                )

            def fin_scale_strip(s):
                zbf = zbfp.tile([128, 4, D], bf16, tag="zbf", name=f"zbf{s}")
                strip_state[(s, "zbf")] = zbf
                for k in range(4):
                    t = s * 4 + k
                    nc.vector.tensor_scalar_mul(
                        zbf[:, k, :], zraw[:, t, :], rinorm[:, t : t + 1]
                    )
                tp = psum_tp.tile([128, 1024], bf16, tag="tp", name=f"tp{s}")
                strip_state[(s, "tp")] = tp
                for d in range(2):
                    for k in range(4):
                        nc.tensor.transpose(
                            tp[:, (d * 4 + k) * 128 : (d * 4 + k + 1) * 128],
                            zbf[:, k, d * 128 : (d + 1) * 128],
                            ident,
                        )

            def fin_copy_strip(s):
                tp = strip_state[(s, "tp")]
                nc.vector.tensor_copy(
                    zT8[:, :, s * 512 : s * 512 + 512],
                    tp.rearrange("p (d c) -> p d c", d=2),
                )

            # ---------------- main-phase helpers ----------------
            strip_state = {}

            def main_batch(sl, c):
                off, W = CHUNKS[c]
                jb = sl * 512 + off
                if c == 0:
                    et = exppool.tile([128, 4, JSPAN], expdt, tag="exp8")
                    strip_state[sl] = et
                et = strip_state[sl]

                for it in range(4):
                    ig = sl * 4 + it
                    pt = psum.tile([128, 1536], f32, tag="pt")
                    for jc in range(W // 512):
                        j0 = jb + jc * 512
                        nc.tensor.matmul(
                            pt[:, jc * 512 : (jc + 1) * 512],
                            lhsT=zT8[:, :, ig * 128 : (ig + 1) * 128],
                            rhs=zT8[:, :, j0 : j0 + 512],
                            start=True,
                            stop=True,
                            perf_mode=DR,
                            skip_group_check=True,
                        )
                    nc.scalar.activation(
                        out=et[:, it, off : off + W],
                        in_=pt[:, 0:W],
                        func=Act.Exp,
                        scale=float(INV_TEMP / (ZSCALE * ZSCALE)),
                        accum_out=accum[:, ig * NCHUNK + c : ig * NCHUNK + c + 1],
                    )

                    if c == 2 and it in (1, 3):
                        # column sums for this it-pair (k=1..7, fp8
                        # DoubleRow), accumulated in a tp-pool bank
                        pair = it // 2
                        if pair == 0:
                            strip_state[(sl, "cp")] = psum_tp.tile(
                                [8, 512], f32, tag="tp", name=f"cp{sl}"
                            )
                        cp = strip_state[(sl, "cp")]
                        for k in range(1, KDIST):
                            nc.tensor.matmul(
                                cp[:, :],
                                lhsT=oh[:, :, k - 1, :],
                                rhs=et[:, 2 * pair : 2 * pair + 2,
                                       k * 512 : (k + 1) * 512],
                                start=(k == 1 and pair == 0),
                                stop=(k == KDIST - 1 and pair == 1),
                                perf_mode=DR,
                                skip_group_check=True,
                            )
                        if pair == 1:
                            # stage to SBUF (DMA can't read PSUM)
                            nc.vector.tensor_copy(colsb[:, sl, :], cp[:, :])
                            nc.sync.dma_start(
                                out=scol_dram[:, sl, :], in_=colsb[:, sl, :]
                            )
                            nc.sync.dma_start(
                                out=s_dram[:, sl * 12 : sl * 12 + 12],
                                in_=accum[:, sl * 12 : sl * 12 + 12],
                            )

            def pos_pairs():
                # positive pairs: raw dot of slab rows (tiles 0..7) with their
                # partner rows at +B (tiles 32..39)
                for t in range(8):
                    pscr = scratch.tile([128, D], bf16, tag="sq_scratch")
                    nc.vector.scalar_tensor_tensor(
                        out=pscr,
                        in0=zraw[:, t, :],
                        scalar=1.0,
                        in1=zraw[:, t + 32, :],
                        op0=Alu.mult,
                        op1=Alu.mult,
                        accum_out=dotraw[:, t : t + 1],
                    )
                nc.vector.tensor_mul(tmp8, rinorm[:, 0:8], rinorm[:, 32:40])
                nc.vector.scalar_tensor_tensor(
                    out=pos2,
                    in0=dotraw,
                    scalar=float(INV_TEMP / (ZSCALE * ZSCALE)),
                    in1=tmp8,
                    op0=Alu.mult,
                    op1=Alu.mult,
                )
                nc.sync.dma_start(out=p2_dram[:, :], in_=pos2)

            # ---------------- emission schedule ----------------
            # DMAs all up front (SP-HWDGE stream); squares on Pool for strips
            # 0..7 (up front), on DVE for strips 8,9 (emitted late so they
            # don't block the DVE stream on their DMAs).  Norms just-in-time
            # in the ACT stream; fin transposes all BEFORE the ACT-paced main
            # batches so the PE stream never parks load work behind them.
            for s in range(NLOAD_S):
                load_dma(s)
            for s in range(NLOAD_S):
                strip_sq(s, nc.vector)
            sched = [
                ("N", 0, 4), ("N", 4, 8), ("S", 0), ("S", 1),
                ("N", 8, 12), ("S", 2), ("C", 0), ("C", 1), ("C", 2),
                ("B", 0, 0),
                ("N", 12, 16), ("S", 3), ("C", 3), ("B", 1, 0),
                ("N", 16, 24), ("S", 4), ("S", 5), ("C", 4), ("C", 5),
                ("N", 24, 32), ("S", 6), ("S", 7), ("C", 6), ("C", 7),
                ("N", 32, 40), ("S", 8), ("S", 9), ("C", 8), ("C", 9),
                ("B", 0, 1), ("B", 1, 1), ("B", 0, 2),
                ("P",),
                ("B", 1, 2),
            ]
            for item in sched:
                if item[0] == "N":
                    norm_strips(item[1], item[2])
                elif item[0] == "S":
                    fin_scale_strip(item[1])
                elif item[0] == "C":
                    fin_copy_strip(item[1])
                elif item[0] == "B":
                    main_batch(item[1], item[2])
                elif item[0] == "P":
                    pos_pairs()


    return nc


def _get_nc():
    global _nc_cache
    if _nc_cache is None:
        _nc_cache = _build_nc()
    return _nc_cache


def kernel(x_i, x_j):
    from concourse import bass_utils

    z = np.concatenate(
        [np.asarray(x_i, dtype=np.float32), np.asarray(x_j, dtype=np.float32)], axis=0
    )
    in_maps = [
        {"z": np.ascontiguousarray(
            np.roll(z, -c * SLAB, axis=0)[: NLOAD_S * STRIP]
        ).astype(ml_dtypes.bfloat16)}
        for c in range(NCORES)
    ]
    nc = _get_nc()
    res = bass_utils.run_bass_kernel_spmd(nc, in_maps, core_ids=list(range(NCORES)))

    S_glob = np.zeros(NB, dtype=np.float64)
    P2_glob = np.zeros(NB, dtype=np.float64)
    for c in range(NCORES):
        acc = np.asarray(res.results[c]["s_out"], dtype=np.float64).reshape(
            128, 8, NCHUNK
        )
        srow = acc.sum(axis=2)                                         # [128, 8]
        p2 = np.asarray(res.results[c]["p2_out"], dtype=np.float64)    # [128, 8]
        scol = np.asarray(res.results[c]["scol_out"], dtype=np.float64)  # [8, 2, 512]
        base = c * SLAB
        S_glob[base : base + SLAB] += srow.T.reshape(SLAB)
        P2_glob[base : base + SLAB] = p2.T.reshape(SLAB)
        for sl in range(2):
            for k in range(1, KDIST):
                t = (2 * c + sl + k) % NSTRIP
                S_glob[t * STRIP : (t + 1) * STRIP] += scol[k - 1, sl]

    loss = -P2_glob + np.log(S_glob - E2 + np.exp(P2_glob))
    return np.array(loss.mean(), dtype=np.float32)


# revision 19
# speedup vs baseline: 1.5481x; 1.2304x over previous
"""SimCLR-style contrastive loss (nn_Contrast) on 8 Trainium2 NeuronCores.

Symmetry-exploiting data-parallel scheme:
  z = concat(normalize(x_i), normalize(x_j)) has a SYMMETRIC sim matrix
  sim = (z @ z.T)/TEMP, so each (i,j) block only needs to be computed once
  globally.  Rows are split into 16 strips of 512; the core owning strips
  (2c, 2c+1) (host pre-rotates z by -c*1024 so every core sees its strips
  as local strips 0,1 -- one SPMD program) computes, for each of its
  strips s, the blocks (s, s+k) for k=0..8, with the k=8 block
  half-weighted (exp bias = -ln2; the distance-8 pair is computed by both
  covering cores).  Per core that is 2*8.5 = 17 block-equivalents of
  512x512 instead of the 64 a full slab costs -- 47% less exp/matmul work.

  Row sums of exp come free via the activation's accum_out.  The missing
  contributions (each block also serves the rows of its COLUMN strip) are
  computed as column sums on the PE: the exp'd block is written to SBUF
  (fp8) and contracted against one-hot ones-vectors with a DoubleRow fp8
  matmul into a [8, 512] PSUM tile (partition = k), DMA'd out, and the
  host adds them into the right global strips.

  Device outputs per core: s_out [128,8] row-sum partials, scol [16,512]
  column-sum vectors (2 strips x k=1..8), p2_out [128,8] positive-pair
  logits.  Host: S = scatter(srow) + scatter(scol); loss =
  mean(-p2 + log(S - e^2 + exp(p2))).
"""

import numpy as np
import ml_dtypes

B = 4096
D = 256
NB = 2 * B              # 8192 rows of z
NCORES = 8
SLAB = NB // NCORES     # 1024 rows per core
STRIP = 512
NSTRIP = NB // STRIP    # 16 strips globally
KDIST = 8               # max block distance (k=8 half-weighted)
NLOAD_S = 2 + KDIST     # strips loaded per core (own 2 + cols up to k=8 from strip 1)
NLOAD_T = NLOAD_S * 4   # 40 row tiles of 128
JSPAN = (KDIST + 1) * STRIP  # 4608 j-columns per strip
NCHUNK = 3              # j-chunks per strip: 3 x 1536
CHUNKS = [(0, 1536), (1536, 1536), (3072, 1536)]
CHUNK_KS = [(1, 2), (3, 4, 5), (6, 7)]  # k-strips needing colsums per chunk
# (k=8 blocks are computed redundantly by both covering cores at full weight;
#  each strip's k=8 contribution comes from its own rowsums, so no k=8
#  colsums and no half-weighting are needed.)
ZSCALE = 16.0           # fp8 pre-scale of normalized z rows
LN_ZSCALE = float(np.log(ZSCALE))
TEMP = 0.5
INV_TEMP = 1.0 / TEMP
E2 = float(np.exp(INV_TEMP))

_nc_cache = None


def _patch_tile_drain():
    """This container's walrus accepts at most ONE sem-wait per instruction,
    but Tile's wait assignment can attach several (and the tail drain gets
    one per busy proc).  Legalize by hoisting extra waits onto preceding
    same-engine NoOps (same semantics: an engine executes its stream in
    order, and multi-waits are AND conditions)."""
    import concourse.tile as tile
    from concourse import mybir
    from concourse.vector_clock import ScopedClock

    if getattr(tile.TileContext, "_drain_patch_applied", False):
        return

    _ctr = [0]

    def _legalize_waits(nc):
        for f in nc.m.functions:
            for bb in f.blocks:
                insts = bb.instructions
                new = []
                changed = False
                for inst in insts:
                    si = inst.sync_info
                    waits = list(si.on_wait) if (si and si.on_wait) else []
                    if len(waits) > 1:
                        for w in waits[:-1]:
                            _ctr[0] += 1
                            nop = mybir.InstNoOp(
                                name=f"legalize-wait-{_ctr[0]}", ins=[], outs=[]
                            )
                            nop.engine = inst.engine
                            nop.sync_info = mybir.SyncInfo(
                                on_wait=[w], on_update=[]
                            )
                            new.append(nop)
                        si.on_wait = [waits[-1]]
                        changed = True
                    new.append(inst)
                if changed:
                    bb.instructions = new

    def _drain_and_barrier(self, tick_clock, wait_clock):
        nc = self.nc
        nop0 = nc.sync.nop()
        wait_clock.add_sem_waits(
            nop0.ins, ScopedClock({None: tick_clock.global_clock})
        )
        nc.sync.drain()
        nc.all_engine_barrier()
        assert self.sems is not None
        popped = nc._tile_sem_poison_stack.pop()
        assert popped is self._sem_poison
        nc.clear_and_free_semaphores(list(self.sems.allocated().values()))
        nc.all_engine_barrier()
        _legalize_waits(nc)

    tile.TileContext._drain_and_barrier = _drain_and_barrier
    tile.TileContext._drain_patch_applied = True


def _build_nc(repeat=1, exp_fp8=True):
    """repeat=1: single body.  repeat=2k: hardware loop of k iterations, each
    containing TWO software-pipelined bodies on alternating buffer sets so
    consecutive bodies overlap (body B's load phase hides under body A's
    ACT-bound main phase)."""
    from concourse import mybir, masks
    import concourse.bass as bass
    import concourse.tile as tile
    import contextlib

    _patch_tile_drain()

    f32 = mybir.dt.float32
    bf16 = mybir.dt.bfloat16
    fp8 = mybir.dt.float8e4
    expdt = fp8 if exp_fp8 else bf16
    Act = mybir.ActivationFunctionType
    Alu = mybir.AluOpType
    DR = mybir.MatmulPerfMode.DoubleRow

    assert repeat == 1 or repeat % 2 == 0
    nbodies = 1 if repeat == 1 else 2

    nc = bass.Bass()
    z_dram = nc.dram_tensor("z", [NLOAD_S * STRIP, D], bf16, kind="ExternalInput")
    s_dram = nc.dram_tensor("s_out", [128, 8 * NCHUNK], f32, kind="ExternalOutput")
    p2_dram = nc.dram_tensor("p2_out", [128, 8], f32, kind="ExternalOutput")
    scol_dram = nc.dram_tensor("scol_out", [8, 2, 512], f32, kind="ExternalOutput")

    with tile.TileContext(nc) as tc:
        rep_ctx = (
            tc.For_i(0, repeat // 2) if repeat > 1 else contextlib.nullcontext()
        )
        with (
            rep_ctx,
            tc.tile_pool(name="persist", bufs=1) as persist,
            tc.tile_pool(name="scratch", bufs=4) as scratch,
            tc.tile_pool(name="zbfp", bufs=2) as zbfp,
            tc.tile_pool(name="exppool", bufs=2) as exppool,
            tc.tile_pool(name="psum", bufs=2, space="PSUM") as psum,
            tc.tile_pool(name="psum_tp", bufs=2, space="PSUM") as psum_tp,
        ):
            ident = persist.tile([128, 128], bf16, tag="ident")
            bln16 = persist.tile([128, 1], f32, tag="bln16")
            # one-hot ones weights for column-sum matmuls: [kt, idx, m]
            oh = persist.tile([128, 2, KDIST, KDIST], expdt, tag="oh")

            masks.make_identity(nc, ident[:])
            nc.vector.memset(bln16, LN_ZSCALE)
            nc.vector.memset(oh, 0.0)
            for kt in range(2):
                for idx in range(KDIST):
                    nc.vector.memset(oh[:, kt, idx, idx : idx + 1], 1.0)

            def emit_body(b):
                zraw = persist.tile(
                    [128, NLOAD_T, D], bf16, tag=f"zraw{b}", name=f"zraw{b}"
                )
                zT8 = persist.tile(
                    [128, 2, NLOAD_S * STRIP], fp8, tag=f"zT8{b}", name=f"zT8{b}"
                )
                norms2 = persist.tile(
                    [128, NLOAD_T], f32, tag=f"norms2{b}", name=f"norms2{b}"
                )
                lnb = persist.tile(
                    [128, NLOAD_T], f32, tag=f"lnb{b}", name=f"lnb{b}"
                )
                rinorm = persist.tile(
                    [128, NLOAD_T], f32, tag=f"rinorm{b}", name=f"rinorm{b}"
                )
                accum = persist.tile(
                    [128, 8 * NCHUNK], f32, tag=f"accum{b}", name=f"accum{b}"
                )
                dotraw = persist.tile(
                    [128, 8], f32, tag=f"dotraw{b}", name=f"dotraw{b}"
                )
                tmp8 = persist.tile([128, 8], f32, tag=f"tmp8{b}", name=f"tmp8{b}")
                pos2 = persist.tile([128, 8], f32, tag=f"pos2{b}", name=f"pos2{b}")
                colsb = persist.tile(
                    [8, 2, 512], f32, tag=f"colsb{b}", name=f"colsb{b}"
                )
                strip_state = {}

                def load_dma(s):
                    nc.sync.dma_start(
                        out=zraw[:, s * 4 : (s + 1) * 4, :],
                        in_=z_dram[s * 512 : (s + 1) * 512, :].rearrange(
                            "(t p) d -> p t d", p=128
                        ),
                    )

                def strip_sq(s):
                    for k in range(4):
                        t = s * 4 + k
                        sq = scratch.tile([128, D], bf16, tag="sq_scratch")
                        nc.vector.scalar_tensor_tensor(
                            out=sq,
                            in0=zraw[:, t, :],
                            scalar=1.0,
                            in1=zraw[:, t, :],
                            op0=Alu.mult,
                            op1=Alu.mult,
                            accum_out=norms2[:, t : t + 1],
                        )

                def norm_strips(t0, t1):
                    # rinorm = ZSCALE * exp(-0.5 * ln(sumsq)), tiles [t0, t1)
                    gs = slice(t0, t1)
                    nc.scalar.activation(
                        out=lnb[:, gs], in_=norms2[:, gs], func=Act.Ln
                    )
                    nc.scalar.activation(
                        out=rinorm[:, gs], in_=lnb[:, gs], func=Act.Exp,
                        scale=-0.5, bias=bln16[:, :],
                    )

                def fin_scale_strip(s):
                    zbf = zbfp.tile(
                        [128, 4, D], bf16, tag="zbf", name=f"zbf{b}_{s}"
                    )
                    strip_state[(s, "zbf")] = zbf
                    for k in range(4):
                        t = s * 4 + k
                        nc.vector.tensor_scalar_mul(
                            zbf[:, k, :], zraw[:, t, :], rinorm[:, t : t + 1]
                        )
                    tp = psum_tp.tile(
                        [128, 1024], bf16, tag="tp", name=f"tp{b}_{s}"
                    )
                    strip_state[(s, "tp")] = tp
                    for d in range(2):
                        for k in range(4):
                            nc.tensor.transpose(
                                tp[:, (d * 4 + k) * 128 : (d * 4 + k + 1) * 128],
                                zbf[:, k, d * 128 : (d + 1) * 128],
                                ident,
                            )

                def fin_copy_strip(s):
                    tp = strip_state[(s, "tp")]
                    nc.vector.tensor_copy(
                        zT8[:, :, s * 512 : s * 512 + 512],
                        tp.rearrange("p (d c) -> p d c", d=2),
                    )

                def main_batch(sl, c):
                    off, W = CHUNKS[c]
                    jb = sl * 512 + off
                    if c == 0:
                        strip_state[sl] = exppool.tile(
                            [128, 4, JSPAN], expdt, tag="exp8", name=f"et{b}_{sl}"
                        )
                    et = strip_state[sl]

                    for it in range(4):
                        ig = sl * 4 + it
                        pt = psum.tile([128, 1536], f32, tag="pt")
                        for jc in range(W // 512):
                            j0 = jb + jc * 512
                            nc.tensor.matmul(
                                pt[:, jc * 512 : (jc + 1) * 512],
                                lhsT=zT8[:, :, ig * 128 : (ig + 1) * 128],
                                rhs=zT8[:, :, j0 : j0 + 512],
                                start=True,
                                stop=True,
                                perf_mode=DR,
                                skip_group_check=True,
                            )
                        nc.scalar.activation(
                            out=et[:, it, off : off + W],
                            in_=pt[:, 0:W],
                            func=Act.Exp,
                            scale=float(INV_TEMP / (ZSCALE * ZSCALE)),
                            accum_out=accum[
                                :, ig * NCHUNK + c : ig * NCHUNK + c + 1
                            ],
                        )

                        if c == 2 and it in (1, 3):
                            # column sums for this it-pair (k=1..7, fp8
                            # DoubleRow), accumulated in a tp-pool bank
                            pair = it // 2
                            if pair == 0:
                                strip_state[(sl, "cp")] = psum_tp.tile(
                                    [8, 512], f32, tag="tp", name=f"cp{b}_{sl}"
                                )
                            cp = strip_state[(sl, "cp")]
                            for k in range(1, KDIST):
                                nc.tensor.matmul(
                                    cp[:, :],
                                    lhsT=oh[:, :, k - 1, :],
                                    rhs=et[:, 2 * pair : 2 * pair + 2,
                                           k * 512 : (k + 1) * 512],
                                    start=(k == 1 and pair == 0),
                                    stop=(k == KDIST - 1 and pair == 1),
                                    perf_mode=DR,
                                    skip_group_check=True,
                                )
                            if pair == 1:
                                # stage to SBUF (DMA can't read PSUM)
                                nc.vector.tensor_copy(colsb[:, sl, :], cp[:, :])
                                nc.sync.dma_start(
                                    out=scol_dram[:, sl, :], in_=colsb[:, sl, :]
                                )
                                nc.sync.dma_start(
                                    out=s_dram[:, sl * 12 : sl * 12 + 12],
                                    in_=accum[:, sl * 12 : sl * 12 + 12],
                                )

                def pos_pairs():
                    # positive pairs: raw dot of slab rows (tiles 0..7) with
                    # their partner rows at +B (tiles 32..39)
                    for t in range(8):
                        pscr = scratch.tile([128, D], bf16, tag="sq_scratch")
                        nc.vector.scalar_tensor_tensor(
                            out=pscr,
                            in0=zraw[:, t, :],
                            scalar=1.0,
                            in1=zraw[:, t + 32, :],
                            op0=Alu.mult,
                            op1=Alu.mult,
                            accum_out=dotraw[:, t : t + 1],
                        )
                    nc.vector.tensor_mul(tmp8, rinorm[:, 0:8], rinorm[:, 32:40])
                    nc.vector.scalar_tensor_tensor(
                        out=pos2,
                        in0=dotraw,
                        scalar=float(INV_TEMP / (ZSCALE * ZSCALE)),
                        in1=tmp8,
                        op0=Alu.mult,
                        op1=Alu.mult,
                    )
                    nc.sync.dma_start(out=p2_dram[:, :], in_=pos2)

                # ---- emission schedule: DMAs up front; squares, norms,
                # scale/transpose, copies just-in-time per engine stream so
                # nothing early in a stream waits on a late dependency.
                for s in range(NLOAD_S):
                    load_dma(s)
                sched = [
                    ("Q", 0), ("Q", 1), ("N", 0, 8), ("S", 0), ("S", 1),
                    ("Q", 2), ("N", 8, 12), ("S", 2),
                    ("C", 0), ("C", 1), ("C", 2), ("B", 0, 0),
                    ("Q", 3), ("N", 12, 16), ("S", 3), ("C", 3), ("B", 1, 0),
                    ("Q", 4), ("Q", 5), ("N", 16, 24), ("S", 4), ("S", 5),
                    ("C", 4), ("C", 5),
                    ("Q", 6), ("Q", 7), ("N", 24, 32), ("S", 6), ("S", 7),
                    ("C", 6), ("C", 7),
                    ("Q", 8), ("Q", 9), ("N", 32, 40), ("S", 8), ("S", 9),
                    ("C", 8), ("C", 9),
                    ("B", 0, 1), ("B", 1, 1), ("B", 0, 2),
                    ("P",),
                    ("B", 1, 2),
                ]
                for item in sched:
                    if item[0] == "Q":
                        strip_sq(item[1])
                    elif item[0] == "N":
                        norm_strips(item[1], item[2])
                    elif item[0] == "S":
                        fin_scale_strip(item[1])
                    elif item[0] == "C":
                        fin_copy_strip(item[1])
                    elif item[0] == "B":
                        main_batch(item[1], item[2])
                    elif item[0] == "P":
                        pos_pairs()

            for b in range(nbodies):
                emit_body(b)

    return nc


def _get_nc():
    global _nc_cache
    if _nc_cache is None:
        _nc_cache = _build_nc()
    return _nc_cache


def kernel(x_i, x_j):
    from concourse import bass_utils

    z = np.concatenate(
        [np.asarray(x_i, dtype=np.float32), np.asarray(x_j, dtype=np.float32)], axis=0
    )
    in_maps = [
        {"z": np.ascontiguousarray(
            np.roll(z, -c * SLAB, axis=0)[: NLOAD_S * STRIP]
        ).astype(ml_dtypes.bfloat16)}
        for c in range(NCORES)
    ]
    nc = _get_nc()
    res = bass_utils.run_bass_kernel_spmd(nc, in_maps, core_ids=list(range(NCORES)))

    S_glob = np.zeros(NB, dtype=np.float64)
    P2_glob = np.zeros(NB, dtype=np.float64)
    for c in range(NCORES):
        acc = np.asarray(res.results[c]["s_out"], dtype=np.float64).reshape(
            128, 8, NCHUNK
        )
        srow = acc.sum(axis=2)                                         # [128, 8]
        p2 = np.asarray(res.results[c]["p2_out"], dtype=np.float64)    # [128, 8]
        scol = np.asarray(res.results[c]["scol_out"], dtype=np.float64)  # [8, 2, 512]
        base = c * SLAB
        S_glob[base : base + SLAB] += srow.T.reshape(SLAB)
        P2_glob[base : base + SLAB] = p2.T.reshape(SLAB)
        for sl in range(2):
            for k in range(1, KDIST):
                t = (2 * c + sl + k) % NSTRIP
                S_glob[t * STRIP : (t + 1) * STRIP] += scol[k - 1, sl]

    loss = -P2_glob + np.log(S_glob - E2 + np.exp(P2_glob))
    return np.array(loss.mean(), dtype=np.float32)
